# revision 26
# baseline (speedup 1.0000x reference)
"""Cross-attention block on 8 Trainium2 NeuronCores (Bass/Tile).

Reference computation (all f32):
    func_val = func_embed @ Wv_w.T + Wv_b          # [8192, 256]
    z        = (sent_embed @ func_embed.T) / 16    # [16384, 8192]
    out      = softmax(z, axis=1) @ func_val       # [16384, 256]

Sharding: sent_embed rows split across 8 cores (2048 rows each);
func_embed / Wv replicated. Each core runs an identical flash-style
fused kernel:

  * scores are computed TRANSPOSED (zT[k, q]) so that e = exp(zT) is
    already in the [contraction, out-partition] layout the second
    matmul needs as its stationary operand — no on-device transposes.
  * softmax needs no max subtraction here (z ~ N(0,1), exp is safe in
    f32 range) and the denominator falls out of the same matmul by
    augmenting V with a ones column: U = e.T @ [V | 1], then
    out = U[:, :256] * (1 / U[:, 256]).

Host side only reshapes/transposes/casts (layout prep + sharding);
every FLOP of the model runs on device.
"""

import numpy as np

import concourse.bass as bass  # noqa: F401  (bass types used via tile/bacc)
import concourse.tile as tile
from concourse import bacc, mybir
from concourse.bass_utils import run_bass_kernel_spmd

N_CORES = 8
N_FUNC = 8192
N_SENT = 16384
D = 256
QROWS = N_SENT // N_CORES          # 2048 query rows per core
QB = 512                           # query block per flash iteration
NKT = N_FUNC // 128                # 64 key tiles
NV = D + 1                         # V augmented with ones column

WT_DT = mybir.dt.bfloat16          # stationary operands (ft, e)
MV_DT = mybir.dt.bfloat16          # moving operands (qt, wvt, v)
WT_NP = mybir.dt.np(WT_DT)
MV_NP = mybir.dt.np(MV_DT)

_COMPILED = None


def build():
    nc = bacc.Bacc("TRN2", target_bir_lowering=False, debug=False,
                   num_devices=N_CORES)
    f32 = mybir.dt.float32
    ft = nc.dram_tensor("ft", [2, 128, N_FUNC], WT_DT, kind="ExternalInput").ap()
    qt = nc.dram_tensor("qt", [2, 128, QROWS], MV_DT, kind="ExternalInput").ap()
    wvt = nc.dram_tensor("wvt", [2, 128, NV], MV_DT, kind="ExternalInput").ap()
    bias = nc.dram_tensor("bias", [128, D], f32, kind="ExternalInput").ap()
    out = nc.dram_tensor("out", [QROWS, D], f32, kind="ExternalOutput").ap()

    EXP = mybir.ActivationFunctionType.Exp

    # ft is loaded in FTP pieces (along the key axis) so compute can begin
    # as soon as the first piece lands; qt in per-q-block pieces likewise.
    FTP = 1024                     # ft piece width (key columns)
    NFP = N_FUNC // FTP            # 8 pieces
    KPP = FTP // 128               # key tiles per piece

    with tile.TileContext(nc) as tc:
        with (
            tc.tile_pool(name="singles", bufs=1) as singles,
            tc.tile_pool(name="epool", bufs=4) as epool,
            tc.tile_pool(name="opool", bufs=4) as opool,
            tc.tile_pool(name="small", bufs=8) as small,
            tc.tile_pool(name="pz", bufs=3, space="PSUM") as pz,
            tc.tile_pool(name="pu", bufs=1, space="PSUM") as pu,
        ):
            wvt_sb = singles.tile([128, 2, NV], MV_DT)
            bias_sb = singles.tile([128, D], f32)
            v_sb = singles.tile([128, NKT, NV], MV_DT)
            # ones column of V' (the softmax-denominator trick) is static
            nc.vector.memset(v_sb[:, :, D:D + 1], 1.0)

            # warm-up matmuls: keep PE busy (and HAM un-throttled) while the
            # input DMAs land; they accumulate garbage into a scratch bank
            # that is never read.
            dw_t = singles.tile([128, 128], WT_DT)
            dm_t = singles.tile([128, QB], WT_DT)
            nc.vector.memset(dw_t, 0)
            nc.vector.memset(dm_t, 0)
            NDUMMY = 6
            pd_t = pz.tile([128, QB], f32, tag="pz", name="pdummy")
            for i in range(NDUMMY):
                nc.tensor.matmul(pd_t, lhsT=dw_t, rhs=dm_t,
                                 start=(i == 0), stop=(i == NDUMMY - 1))

            # descriptor issue is ~0.65us each per engine queue; only SP and
            # ACT drive the fast HWDGE path. ACT gets only the three most
            # urgent descriptors (its stream must be free for exps); SP takes
            # the rest in priority order. qt blocks 1-3 are issued later, from
            # inside earlier flash blocks.
            NQB = QROWS // QB
            NSUB = QB // 128
            ftp = [singles.tile([128, 2, FTP], WT_DT, name=f"ftp{p}")
                   for p in range(NFP)]
            qtp = [singles.tile([128, 2, QB], MV_DT, name=f"qtp{b}")
                   for b in range(NQB)]

            def ft_dma(eng, p, c):
                eng.dma_start(out=ftp[p][:, c, :],
                              in_=ft[c, :, p * FTP:(p + 1) * FTP])

            def qt_dma(eng, b, c):
                eng.dma_start(out=qtp[b][:, c, :],
                              in_=qt[c, :, b * QB:(b + 1) * QB])

            nc.scalar.dma_start(out=wvt_sb[:, 1, :], in_=wvt[1])
            ft_dma(nc.scalar, 0, 1)
            qt_dma(nc.scalar, 0, 0)
            nc.sync.dma_start(out=wvt_sb[:, 0, :], in_=wvt[0])
            ft_dma(nc.sync, 0, 0)
            qt_dma(nc.sync, 0, 1)
            for p in range(1, NFP):
                ft_dma(nc.sync, p, 0)
                (ft_dma(nc.scalar, p, 1) if p <= 3 else ft_dma(nc.sync, p, 1))
            nc.sync.dma_start(out=bias_sb, in_=bias)

            def ft_sl(k, c):
                return ftp[k // KPP][:, c, (k % KPP) * 128:(k % KPP + 1) * 128]

            # ---- V' = [func_embed @ Wv_w.T | 1]  (per 128-row tile) ---------
            # Wv_b is NOT added here: softmax rows sum to 1, so the bias is
            # folded into the final normalize (out = U/s + b), 4x fewer adds.
            # V' chunks are produced just-in-time inside flash block 0: chunk
            # k is computed two k-steps before its consumer u(k), so the whole
            # projection hides inside the flash pipeline.
            def v_mm(k):
                pv_t = pz.tile([128, NV], f32, tag="pv", bufs=1, name=f"pv{k}")
                nc.tensor.matmul(pv_t, lhsT=ft_sl(k, 0), rhs=wvt_sb[:, 0, :],
                                 start=True, stop=False)
                nc.tensor.matmul(pv_t, lhsT=ft_sl(k, 1), rhs=wvt_sb[:, 1, :],
                                 start=False, stop=True)
                nc.vector.tensor_copy(v_sb[:, k, :D], pv_t[:, :D])

            # ---- flash loop over query blocks --------------------------------
            for b in range(NQB):

                def z_mm(k):
                    pz_t = pz.tile([128, QB], f32, tag="pz", name=f"pzt{b}_{k}")
                    nc.tensor.matmul(pz_t, lhsT=ft_sl(k, 0),
                                     rhs=qtp[b][:, 0, :], start=True, stop=False)
                    nc.tensor.matmul(pz_t, lhsT=ft_sl(k, 1),
                                     rhs=qtp[b][:, 1, :], start=False, stop=True)
                    return pz_t

                pu_ts = [pu.tile([128, NV], f32, tag=f"pu{s}", name=f"pu{s}_{b}")
                         for s in range(NSUB)]
                # software-pipelined by two k so ACT's exp(k) (~0.7us) hides
                # behind PE's z(k+1) + z(k+2) (~0.85us)
                if b == 0:
                    v_mm(0)
                    v_mm(1)
                zq = [z_mm(0), z_mm(1)]
                for k in range(NKT):
                    e_t = epool.tile([128, QB], WT_DT)
                    nc.scalar.activation(e_t, zq.pop(0), EXP, scale=1.0 / 16.0)
                    if k + 2 < NKT:
                        zq.append(z_mm(k + 2))
                        if b == 0:
                            v_mm(k + 2)
                    # late input descriptors on idle SP: qt for blocks 1-3
                    if k == 40 and b < NQB - 1:
                        qt_dma(nc.sync, b + 1, 0)
                        qt_dma(nc.sync, b + 1, 1)
                    for s in range(NSUB):
                        nc.tensor.matmul(pu_ts[s],
                                         lhsT=e_t[:, s * 128:(s + 1) * 128],
                                         rhs=v_sb[:, k, :],
                                         start=(k == 0), stop=(k == NKT - 1))

                last = b == NQB - 1
                for s in range(NSUB):
                    sr = small.tile([128, 1], f32)
                    nc.vector.reciprocal(sr, pu_ts[s][:, D:D + 1])
                    o_t = opool.tile([128, D], f32)
                    if last:
                        # ACT is out of exp work by now; it is faster than DVE
                        # here and runs concurrently with DVE's reciprocals
                        nc.scalar.activation(o_t, pu_ts[s][:, :D],
                                             mybir.ActivationFunctionType.Copy,
                                             scale=sr)
                    else:
                        nc.vector.tensor_scalar_mul(o_t, pu_ts[s][:, :D], sr)
                    nc.vector.tensor_add(o_t, o_t, bias_sb)
                    r0 = b * QB + s * 128
                    deng = nc.sync if s % 2 == 0 else nc.scalar
                    deng.dma_start(out=out[r0:r0 + 128, :], in_=o_t)

    nc.compile()
    return nc


def _prep_inputs(func_embed, sent_embed, Wv_w, Wv_b):
    F = np.ascontiguousarray(np.asarray(func_embed, dtype=np.float32))
    Q = np.ascontiguousarray(np.asarray(sent_embed, dtype=np.float32))
    W = np.asarray(Wv_w, dtype=np.float32)
    b = np.asarray(Wv_b, dtype=np.float32)

    ft_h = np.ascontiguousarray(F.T).reshape(2, 128, N_FUNC).astype(WT_NP)
    wvt_full = np.concatenate([W.T, np.zeros((D, 1), np.float32)], axis=1)
    wvt_h = np.ascontiguousarray(wvt_full).reshape(2, 128, NV).astype(MV_NP)
    bias_h = np.ascontiguousarray(np.broadcast_to(b, (128, D))).astype(np.float32)

    in_maps = []
    for i in range(N_CORES):
        qs = Q[i * QROWS:(i + 1) * QROWS]
        qt_h = np.ascontiguousarray(qs.T).reshape(2, 128, QROWS).astype(MV_NP)
        in_maps.append({"ft": ft_h, "qt": qt_h, "wvt": wvt_h, "bias": bias_h})
    return in_maps


def run(inputs, trace=False, **kw):
    global _COMPILED
    if _COMPILED is None:
        _COMPILED = build()
    in_maps = _prep_inputs(**inputs)
    res = run_bass_kernel_spmd(_COMPILED, in_maps, list(range(N_CORES)),
                               trace=trace, **kw)
    out = np.concatenate([res.results[i]["out"] for i in range(N_CORES)], axis=0)
    return out, res


def kernel(**inputs):
    out, _ = run(inputs, trace=False)
    return out


# revision 28
# speedup vs baseline: 1.0029x; 1.0029x over previous
"""Cross-attention block on 8 Trainium2 NeuronCores (Bass/Tile).

Reference computation (all f32):
    func_val = func_embed @ Wv_w.T + Wv_b          # [8192, 256]
    z        = (sent_embed @ func_embed.T) / 16    # [16384, 8192]
    out      = softmax(z, axis=1) @ func_val       # [16384, 256]

Sharding: sent_embed rows split across 8 cores (2048 rows each);
func_embed / Wv replicated. Each core runs an identical flash-style
fused kernel:

  * scores are computed TRANSPOSED (zT[k, q]) so that e = exp(zT) is
    already in the [contraction, out-partition] layout the second
    matmul needs as its stationary operand — no on-device transposes.
  * softmax needs no max subtraction here (z ~ N(0,1), exp is safe in
    f32 range) and the denominator falls out of the same matmul by
    augmenting V with a ones column: U = e.T @ [V | 1], then
    out = U[:, :256] * (1 / U[:, 256]).

Host side only reshapes/transposes/casts (layout prep + sharding);
every FLOP of the model runs on device.
"""

import numpy as np

import concourse.bass as bass  # noqa: F401  (bass types used via tile/bacc)
import concourse.tile as tile
from concourse import bacc, mybir
from concourse.bass_utils import run_bass_kernel_spmd

N_CORES = 8
N_FUNC = 8192
N_SENT = 16384
D = 256
QROWS = N_SENT // N_CORES          # 2048 query rows per core
QB = 512                           # query block per flash iteration
NKT = N_FUNC // 128                # 64 key tiles
NV = D + 1                         # V augmented with ones column

WT_DT = mybir.dt.bfloat16          # stationary operands (ft, e)
MV_DT = mybir.dt.bfloat16          # moving operands (qt, wvt, v)
WT_NP = mybir.dt.np(WT_DT)
MV_NP = mybir.dt.np(MV_DT)

_COMPILED = None


def build():
    nc = bacc.Bacc("TRN2", target_bir_lowering=False, debug=False,
                   num_devices=N_CORES)
    f32 = mybir.dt.float32
    ft = nc.dram_tensor("ft", [2, 128, N_FUNC], WT_DT, kind="ExternalInput").ap()
    qt = nc.dram_tensor("qt", [2, 128, QROWS], MV_DT, kind="ExternalInput").ap()
    wvt = nc.dram_tensor("wvt", [2, 128, NV], MV_DT, kind="ExternalInput").ap()
    bias = nc.dram_tensor("bias", [128, D], f32, kind="ExternalInput").ap()
    out = nc.dram_tensor("out", [QROWS, D], f32, kind="ExternalOutput").ap()

    EXP = mybir.ActivationFunctionType.Exp

    # ft is loaded in FTP pieces (along the key axis) so compute can begin
    # as soon as the first piece lands; qt in per-q-block pieces likewise.
    FTP = 1024                     # ft piece width (key columns)
    NFP = N_FUNC // FTP            # 8 pieces
    KPP = FTP // 128               # key tiles per piece

    with tile.TileContext(nc) as tc:
        with (
            tc.tile_pool(name="singles", bufs=1) as singles,
            tc.tile_pool(name="epool", bufs=4) as epool,
            tc.tile_pool(name="opool", bufs=4) as opool,
            tc.tile_pool(name="small", bufs=8) as small,
            tc.tile_pool(name="pz", bufs=3, space="PSUM") as pz,
            tc.tile_pool(name="pu", bufs=1, space="PSUM") as pu,
        ):
            wvt_sb = singles.tile([128, 2, NV], MV_DT)
            bias_sb = singles.tile([128, D], f32)
            v_sb = singles.tile([128, NKT, NV], MV_DT)
            # ones column of V' (the softmax-denominator trick) is static
            nc.vector.memset(v_sb[:, :, D:D + 1], 1.0)

            # descriptor issue is ~0.65us each per engine queue; only SP and
            # ACT drive the fast HWDGE path, and ACT's own preamble (exp
            # table + const loads) keeps it busy for the first ~7us — so SP
            # takes every input descriptor, in priority order. qt blocks 1-3
            # are issued later, from inside earlier flash blocks.
            NQB = QROWS // QB
            NSUB = QB // 128
            ftp = [singles.tile([128, 2, FTP], WT_DT, name=f"ftp{p}")
                   for p in range(NFP)]
            qtp = [singles.tile([128, 2, QB], MV_DT, name=f"qtp{b}")
                   for b in range(NQB)]

            def ft_dma(eng, p, c):
                eng.dma_start(out=ftp[p][:, c, :],
                              in_=ft[c, :, p * FTP:(p + 1) * FTP])

            def qt_dma(eng, b, c):
                eng.dma_start(out=qtp[b][:, c, :],
                              in_=qt[c, :, b * QB:(b + 1) * QB])

            nc.sync.dma_start(out=wvt_sb[:, 0, :], in_=wvt[0])
            nc.sync.dma_start(out=wvt_sb[:, 1, :], in_=wvt[1])
            ft_dma(nc.sync, 0, 0)
            ft_dma(nc.sync, 0, 1)
            qt_dma(nc.sync, 0, 0)
            qt_dma(nc.sync, 0, 1)
            for p in range(1, NFP):
                ft_dma(nc.sync, p, 0)
                ft_dma(nc.sync, p, 1)
            nc.sync.dma_start(out=bias_sb, in_=bias)

            def ft_sl(k, c):
                return ftp[k // KPP][:, c, (k % KPP) * 128:(k % KPP + 1) * 128]

            # ---- V' = [func_embed @ Wv_w.T | 1]  (per 128-row tile) ---------
            # Wv_b is NOT added here: softmax rows sum to 1, so the bias is
            # folded into the final normalize (out = U/s + b), 4x fewer adds.
            # V' chunks are produced just-in-time inside flash block 0: chunk
            # k is computed two k-steps before its consumer u(k), so the whole
            # projection hides inside the flash pipeline.
            def v_mm(k):
                pv_t = pz.tile([128, NV], f32, tag="pv", bufs=1, name=f"pv{k}")
                nc.tensor.matmul(pv_t, lhsT=ft_sl(k, 0), rhs=wvt_sb[:, 0, :],
                                 start=True, stop=False)
                nc.tensor.matmul(pv_t, lhsT=ft_sl(k, 1), rhs=wvt_sb[:, 1, :],
                                 start=False, stop=True)
                nc.vector.tensor_copy(v_sb[:, k, :D], pv_t[:, :D])

            # ---- flash loop over query blocks --------------------------------
            for b in range(NQB):

                def z_mm(k):
                    pz_t = pz.tile([128, QB], f32, tag="pz", name=f"pzt{b}_{k}")
                    nc.tensor.matmul(pz_t, lhsT=ft_sl(k, 0),
                                     rhs=qtp[b][:, 0, :], start=True, stop=False)
                    nc.tensor.matmul(pz_t, lhsT=ft_sl(k, 1),
                                     rhs=qtp[b][:, 1, :], start=False, stop=True)
                    return pz_t

                pu_ts = [pu.tile([128, NV], f32, tag=f"pu{s}", name=f"pu{s}_{b}")
                         for s in range(NSUB)]
                # software-pipelined by two k so ACT's exp(k) (~0.7us) hides
                # behind PE's z(k+1) + z(k+2) (~0.85us)
                if b == 0:
                    v_mm(0)
                    v_mm(1)
                zq = [z_mm(0), z_mm(1)]
                for k in range(NKT):
                    e_t = epool.tile([128, QB], WT_DT)
                    nc.scalar.activation(e_t, zq.pop(0), EXP, scale=1.0 / 16.0)
                    if k + 2 < NKT:
                        zq.append(z_mm(k + 2))
                        if b == 0:
                            v_mm(k + 2)
                    # late input descriptors on idle SP: qt for blocks 1-3
                    if k == 40 and b < NQB - 1:
                        qt_dma(nc.sync, b + 1, 0)
                        qt_dma(nc.sync, b + 1, 1)
                    for s in range(NSUB):
                        nc.tensor.matmul(pu_ts[s],
                                         lhsT=e_t[:, s * 128:(s + 1) * 128],
                                         rhs=v_sb[:, k, :],
                                         start=(k == 0), stop=(k == NKT - 1))

                last = b == NQB - 1
                for s in range(NSUB):
                    sr = small.tile([128, 1], f32)
                    nc.vector.reciprocal(sr, pu_ts[s][:, D:D + 1])
                    o_t = opool.tile([128, D], f32)
                    if last:
                        # ACT is out of exp work by now; it is faster than DVE
                        # here and runs concurrently with DVE's reciprocals
                        nc.scalar.activation(o_t, pu_ts[s][:, :D],
                                             mybir.ActivationFunctionType.Copy,
                                             scale=sr)
                    else:
                        nc.vector.tensor_scalar_mul(o_t, pu_ts[s][:, :D], sr)
                    nc.vector.tensor_add(o_t, o_t, bias_sb)
                    r0 = b * QB + s * 128
                    deng = nc.sync if s % 2 == 0 else nc.scalar
                    deng.dma_start(out=out[r0:r0 + 128, :], in_=o_t)

    nc.compile()
    return nc


def _prep_inputs(func_embed, sent_embed, Wv_w, Wv_b):
    F = np.ascontiguousarray(np.asarray(func_embed, dtype=np.float32))
    Q = np.ascontiguousarray(np.asarray(sent_embed, dtype=np.float32))
    W = np.asarray(Wv_w, dtype=np.float32)
    b = np.asarray(Wv_b, dtype=np.float32)

    ft_h = np.ascontiguousarray(F.T).reshape(2, 128, N_FUNC).astype(WT_NP)
    wvt_full = np.concatenate([W.T, np.zeros((D, 1), np.float32)], axis=1)
    wvt_h = np.ascontiguousarray(wvt_full).reshape(2, 128, NV).astype(MV_NP)
    bias_h = np.ascontiguousarray(np.broadcast_to(b, (128, D))).astype(np.float32)

    in_maps = []
    for i in range(N_CORES):
        qs = Q[i * QROWS:(i + 1) * QROWS]
        qt_h = np.ascontiguousarray(qs.T).reshape(2, 128, QROWS).astype(MV_NP)
        in_maps.append({"ft": ft_h, "qt": qt_h, "wvt": wvt_h, "bias": bias_h})
    return in_maps


def run(inputs, trace=False, **kw):
    global _COMPILED
    if _COMPILED is None:
        _COMPILED = build()
    in_maps = _prep_inputs(**inputs)
    res = run_bass_kernel_spmd(_COMPILED, in_maps, list(range(N_CORES)),
                               trace=trace, **kw)
    out = np.concatenate([res.results[i]["out"] for i in range(N_CORES)], axis=0)
    return out, res


def kernel(**inputs):
    out, _ = run(inputs, trace=False)
    return out


# revision 32
# speedup vs baseline: 1.0099x; 1.0069x over previous
"""Cross-attention block on 8 Trainium2 NeuronCores (Bass/Tile).

Reference computation (all f32):
    func_val = func_embed @ Wv_w.T + Wv_b          # [8192, 256]
    z        = (sent_embed @ func_embed.T) / 16    # [16384, 8192]
    out      = softmax(z, axis=1) @ func_val       # [16384, 256]

Sharding: sent_embed rows split across 8 cores (2048 rows each);
func_embed / Wv replicated. Each core runs an identical flash-style
fused kernel:

  * scores are computed TRANSPOSED (zT[k, q]) so that e = exp(zT) is
    already in the [contraction, out-partition] layout the second
    matmul needs as its stationary operand — no on-device transposes.
  * softmax needs no max subtraction here (z ~ N(0,1), exp is safe in
    f32 range) and the denominator falls out of the same matmul by
    augmenting V with a ones column: U = e.T @ [V | 1], then
    out = U[:, :256] * (1 / U[:, 256]).

Host side only reshapes/transposes/casts (layout prep + sharding);
every FLOP of the model runs on device.
"""

import numpy as np

import concourse.bass as bass  # noqa: F401  (bass types used via tile/bacc)
import concourse.tile as tile
from concourse import bacc, mybir
from concourse.bass_utils import run_bass_kernel_spmd

N_CORES = 8
N_FUNC = 8192
N_SENT = 16384
D = 256
QROWS = N_SENT // N_CORES          # 2048 query rows per core
QB = 512                           # query block per flash iteration
NKT = N_FUNC // 128                # 64 key tiles
NV = D + 1                         # V augmented with ones column

WT_DT = mybir.dt.bfloat16          # stationary operands (ft, e)
MV_DT = mybir.dt.bfloat16          # moving operands (qt, wvt, v)
WT_NP = mybir.dt.np(WT_DT)
MV_NP = mybir.dt.np(MV_DT)

_COMPILED = None


def build():
    nc = bacc.Bacc("TRN2", target_bir_lowering=False, debug=False,
                   num_devices=N_CORES)
    f32 = mybir.dt.float32
    ft = nc.dram_tensor("ft", [128, 2, N_FUNC], WT_DT, kind="ExternalInput").ap()
    qt = nc.dram_tensor("qt", [128, 2, QROWS], MV_DT, kind="ExternalInput").ap()
    wvt = nc.dram_tensor("wvt", [128, 2, NV], MV_DT, kind="ExternalInput").ap()
    bias = nc.dram_tensor("bias", [128, D], f32, kind="ExternalInput").ap()
    out = nc.dram_tensor("out", [QROWS, D], f32, kind="ExternalOutput").ap()

    EXP = mybir.ActivationFunctionType.Exp

    # ft is loaded in FTP pieces (along the key axis) so compute can begin
    # as soon as the first piece lands; qt in per-q-block pieces likewise.
    FTP = 1024                     # ft piece width (key columns)
    NFP = N_FUNC // FTP            # 8 pieces
    KPP = FTP // 128               # key tiles per piece

    with tile.TileContext(nc) as tc:
        with (
            tc.tile_pool(name="singles", bufs=1) as singles,
            tc.tile_pool(name="epool", bufs=4) as epool,
            tc.tile_pool(name="opool", bufs=4) as opool,
            tc.tile_pool(name="small", bufs=8) as small,
            tc.tile_pool(name="pz", bufs=3, space="PSUM") as pz,
            tc.tile_pool(name="pu", bufs=1, space="PSUM") as pu,
        ):
            wvt_sb = singles.tile([128, 2, NV], MV_DT)
            bias_sb = singles.tile([128, D], f32)
            v_sb = singles.tile([128, NKT, NV], MV_DT)
            # ones column of V' (the softmax-denominator trick) is static
            nc.vector.memset(v_sb[:, :, D:D + 1], 1.0)

            # descriptor issue is ~0.65us each per engine queue; only SP and
            # ACT drive the fast HWDGE path, and ACT's own preamble (exp
            # table + const loads) keeps it busy for the first ~7us — so SP
            # takes every input descriptor, in priority order. qt blocks 1-3
            # are issued later, from inside earlier flash blocks.
            NQB = QROWS // QB
            NSUB = QB // 128
            ftp = [singles.tile([128, 2, FTP], WT_DT, name=f"ftp{p}")
                   for p in range(NFP)]
            qtp = [singles.tile([128, 2, QB], MV_DT, name=f"qtp{b}")
                   for b in range(NQB)]

            def ft_dma(eng, p):
                eng.dma_start(out=ftp[p],
                              in_=ft[:, :, p * FTP:(p + 1) * FTP])

            def qt_dma(eng, b):
                eng.dma_start(out=qtp[b],
                              in_=qt[:, :, b * QB:(b + 1) * QB])

            nc.sync.dma_start(out=wvt_sb, in_=wvt)
            ft_dma(nc.sync, 0)
            qt_dma(nc.sync, 0)
            for p in range(1, NFP):
                ft_dma(nc.sync, p)
            nc.sync.dma_start(out=bias_sb, in_=bias)

            def ft_sl(k, c):
                return ftp[k // KPP][:, c, (k % KPP) * 128:(k % KPP + 1) * 128]

            # ---- V' = [func_embed @ Wv_w.T | 1]  (per 128-row tile) ---------
            # Wv_b is NOT added here: softmax rows sum to 1, so the bias is
            # folded into the final normalize (out = U/s + b), 4x fewer adds.
            # V' chunks are produced just-in-time inside flash block 0: chunk
            # k is computed two k-steps before its consumer u(k), so the whole
            # projection hides inside the flash pipeline.
            def v_mm(k):
                pv_t = pz.tile([128, NV], f32, tag="pv", bufs=1, name=f"pv{k}")
                nc.tensor.matmul(pv_t, lhsT=ft_sl(k, 0), rhs=wvt_sb[:, 0, :],
                                 start=True, stop=False)
                nc.tensor.matmul(pv_t, lhsT=ft_sl(k, 1), rhs=wvt_sb[:, 1, :],
                                 start=False, stop=True)
                nc.vector.tensor_copy(v_sb[:, k, :D], pv_t[:, :D])

            # ---- flash loop over query blocks --------------------------------
            for b in range(NQB):

                def z_mm(k):
                    pz_t = pz.tile([128, QB], f32, tag="pz", name=f"pzt{b}_{k}")
                    nc.tensor.matmul(pz_t, lhsT=ft_sl(k, 0),
                                     rhs=qtp[b][:, 0, :], start=True, stop=False)
                    nc.tensor.matmul(pz_t, lhsT=ft_sl(k, 1),
                                     rhs=qtp[b][:, 1, :], start=False, stop=True)
                    return pz_t

                pu_ts = [pu.tile([128, NV], f32, tag=f"pu{s}", name=f"pu{s}_{b}")
                         for s in range(NSUB)]
                # software-pipelined by two k so ACT's exp(k) (~0.7us) hides
                # behind PE's z(k+1) + z(k+2) (~0.85us)
                if b == 0:
                    v_mm(0)
                    v_mm(1)
                zq = [z_mm(0), z_mm(1)]
                for k in range(NKT):
                    e_t = epool.tile([128, QB], WT_DT)
                    nc.scalar.activation(e_t, zq.pop(0), EXP, scale=1.0 / 16.0)
                    if k + 2 < NKT:
                        zq.append(z_mm(k + 2))
                        if b == 0:
                            v_mm(k + 2)
                    # late input descriptors on idle SP: qt for blocks 1-3
                    if k == 40 and b < NQB - 1:
                        qt_dma(nc.sync, b + 1)
                    for s in range(NSUB):
                        nc.tensor.matmul(pu_ts[s],
                                         lhsT=e_t[:, s * 128:(s + 1) * 128],
                                         rhs=v_sb[:, k, :],
                                         start=(k == 0), stop=(k == NKT - 1))

                last = b == NQB - 1
                for s in range(NSUB):
                    sr = small.tile([128, 1], f32)
                    nc.vector.reciprocal(sr, pu_ts[s][:, D:D + 1])
                    o_t = opool.tile([128, D], f32)
                    if last:
                        # ACT is out of exp work by now; it is faster than DVE
                        # here and runs concurrently with DVE's reciprocals
                        nc.scalar.activation(o_t, pu_ts[s][:, :D],
                                             mybir.ActivationFunctionType.Copy,
                                             scale=sr)
                    else:
                        nc.vector.tensor_scalar_mul(o_t, pu_ts[s][:, :D], sr)
                    nc.vector.tensor_add(o_t, o_t, bias_sb)
                    r0 = b * QB + s * 128
                    deng = nc.sync if s % 2 == 0 else nc.scalar
                    deng.dma_start(out=out[r0:r0 + 128, :], in_=o_t)

    nc.compile()
    return nc


def _prep_inputs(func_embed, sent_embed, Wv_w, Wv_b):
    F = np.ascontiguousarray(np.asarray(func_embed, dtype=np.float32))
    Q = np.ascontiguousarray(np.asarray(sent_embed, dtype=np.float32))
    W = np.asarray(Wv_w, dtype=np.float32)
    b = np.asarray(Wv_b, dtype=np.float32)

    # device layout [p, c, n]: row p holds both 128-row d-chunks (c=0: d=p,
    # c=1: d=128+p) so each load is a single 3D-strided descriptor
    ft_h = np.ascontiguousarray(
        F.T.reshape(2, 128, N_FUNC).transpose(1, 0, 2)).astype(WT_NP)
    wvt_full = np.concatenate([W.T, np.zeros((D, 1), np.float32)], axis=1)
    wvt_h = np.ascontiguousarray(
        wvt_full.reshape(2, 128, NV).transpose(1, 0, 2)).astype(MV_NP)
    bias_h = np.ascontiguousarray(np.broadcast_to(b, (128, D))).astype(np.float32)

    in_maps = []
    for i in range(N_CORES):
        qs = Q[i * QROWS:(i + 1) * QROWS]
        qt_h = np.ascontiguousarray(
            qs.T.reshape(2, 128, QROWS).transpose(1, 0, 2)).astype(MV_NP)
        in_maps.append({"ft": ft_h, "qt": qt_h, "wvt": wvt_h, "bias": bias_h})
    return in_maps


def run(inputs, trace=False, **kw):
    global _COMPILED
    if _COMPILED is None:
        _COMPILED = build()
    in_maps = _prep_inputs(**inputs)
    res = run_bass_kernel_spmd(_COMPILED, in_maps, list(range(N_CORES)),
                               trace=trace, **kw)
    out = np.concatenate([res.results[i]["out"] for i in range(N_CORES)], axis=0)
    return out, res


def kernel(**inputs):
    out, _ = run(inputs, trace=False)
    return out
